# revision 5
# baseline (speedup 1.0000x reference)
"""Trainium2 Bass kernel for CMPNEncoder functional-group embedding (v5).

out = func_save_init + A @ W,  A[m,:] = sum_a count_m[a] * f_atoms[a,:].

Host prep: compact to the globally-referenced atom rows (~80% of all),
split them evenly across 8 cores, and merge each row's payload into ONE
p-major u8 stream: [133 bf16 feature bytes | 50 nibble-packed count
bytes] = 316 B/row, so every partition's chunk is a single contiguous
multi-KB DMA descriptor.

On device, per 128-row tile: vector/gpsimd unpack the count nibbles to
an fp8 e4m3 [128,100] tile (exact for counts <= 15, host-checked), the
tensor engine contracts counts^T @ features into a PSUM [100,133]
accumulator (fp8 lhsT x bf16 rhs), then A @ W runs on-device.  Host
sums the per-core [100,300] partials and adds func_save_init.
"""

import sys

sys.path.insert(0, "/opt/trn_rl_repo")

import ml_dtypes
import numpy as np

import concourse.bacc as bacc
import concourse.mybir as mybir
from concourse.bass_utils import run_bass_kernel_spmd
from concourse.tile import TileContext

N_ATOMS = 400_000
FDIM = 133
HID = 300
NSEG = 100
N_CORES = 8
CH = 16                                   # 128-row tiles per DMA chunk

FP8 = ml_dtypes.float8_e4m3fn
FBYTES = 2 * FDIM                         # 266 bf16 feature bytes per row


def _chunk_schedule(ntiles):
    """(t0, g) list: small chunks first so the first matmul starts early."""
    sched, t0 = [], 0
    for g in (4, 4, 8):
        if t0 + g > ntiles:
            break
        sched.append((t0, g))
        t0 += g
    while t0 < ntiles:
        g = min(CH, ntiles - t0)
        sched.append((t0, g))
        t0 += g
    return sched


def build_nc(ntiles, nibble=True):
    f32, bf16 = mybir.dt.float32, mybir.dt.bfloat16
    u8, fp8 = mybir.dt.uint8, mybir.dt.float8e4
    rowb = FBYTES + (50 if nibble else NSEG)    # 316 or 366 B per row

    nc = bacc.Bacc("TRN2", target_bir_lowering=False, debug=False)

    mrg = nc.declare_dram_parameter("mrg", [128, ntiles, rowb], u8,
                                    isOutput=False)
    wmat = nc.declare_dram_parameter("wmat", [FDIM, HID], f32, isOutput=False)
    ident_d = nc.declare_dram_parameter("ident", [NSEG, NSEG], f32,
                                        isOutput=False)
    out_d = nc.declare_dram_parameter("out", [NSEG, HID], f32, isOutput=True)

    sched = _chunk_schedule(ntiles)

    with TileContext(nc) as tc:
        with (
            tc.tile_pool(name="const", bufs=1) as cpool,
            tc.tile_pool(name="stream", bufs=8) as spool,
            tc.tile_pool(name="unp", bufs=8) as upool,
            tc.tile_pool(name="psA", bufs=1, space="PSUM") as psA,
            tc.tile_pool(name="psT", bufs=1, space="PSUM") as psT,
            tc.tile_pool(name="sb2", bufs=1) as sb2,
        ):
            st0 = []
            for ck, (t0, g) in enumerate(sched[:2]):
                st = spool.tile([128, CH, rowb], u8, tag="s")
                nc.sync.dma_start(out=st[:, 0:g, :], in_=mrg[:, t0:t0 + g, :])
                st0.append(st)

            ident_t = cpool.tile([NSEG, NSEG], f32, tag="ident")
            nc.sync.dma_start(out=ident_t[:, :], in_=ident_d[:, :])
            wa_t = cpool.tile([128, HID], f32, tag="wa")
            nc.sync.dma_start(out=wa_t[:, :], in_=wmat[0:128, :])
            wb_t = cpool.tile([FDIM - 128, HID], f32, tag="wb")
            nc.sync.dma_start(out=wb_t[:, :], in_=wmat[128:FDIM, :])

            a_ps = psA.tile([NSEG, FDIM], f32, tag="A")

            tglob = 0
            for ck, (t0, g) in enumerate(sched):
                if ck < 2:
                    st = st0[ck]
                else:
                    st = spool.tile([128, CH, rowb], u8, tag="s")
                    nc.sync.dma_start(out=st[:, 0:g, :],
                                      in_=mrg[:, t0:t0 + g, :])
                if nibble:
                    pk = st[:, 0:g, FBYTES:FBYTES + 50]
                    tu = upool.tile([128, CH, NSEG], mybir.dt.uint8, tag="tu")
                    nc.vector.tensor_scalar(
                        out=tu[:, 0:g, 0:50], in0=pk, scalar1=15, scalar2=None,
                        op0=mybir.AluOpType.bitwise_and)
                    nc.vector.tensor_scalar(
                        out=tu[:, 0:g, 50:100], in0=pk, scalar1=4, scalar2=None,
                        op0=mybir.AluOpType.logical_shift_right)
                    wu = upool.tile([128, CH, NSEG], fp8, tag="wu")
                    nc.gpsimd.tensor_scalar(
                        out=wu[:, 0:g, 0:50], in0=tu[:, 0:g, 0:50], scalar1=0,
                        scalar2=None, op0=mybir.AluOpType.add)
                    nc.scalar.activation(
                        out=wu[:, 0:g, 50:100], in_=tu[:, 0:g, 50:100],
                        func=mybir.ActivationFunctionType.Copy,
                        bias=0.0, scale=1.0)
                for j in range(g):
                    if nibble:
                        lhs = wu[:, j, :]
                    else:
                        lhs = st[:, j, FBYTES:FBYTES + NSEG].bitcast(fp8)
                    nc.tensor.matmul(
                        out=a_ps[:, :],
                        lhsT=lhs,
                        rhs=st[:, j, 0:FBYTES].bitcast(bf16),
                        start=(tglob == 0),
                        stop=(tglob == ntiles - 1),
                    )
                    tglob += 1

            a_sb = sb2.tile([NSEG, FDIM], f32, tag="a_sb")
            nc.vector.tensor_copy(out=a_sb[:, :], in_=a_ps[:, :])
            t1_ps = psT.tile([128, NSEG], f32, tag="t1")
            nc.tensor.transpose(out=t1_ps[:, :], in_=a_sb[:, 0:128],
                                identity=ident_t[:, :])
            t2_ps = psT.tile([FDIM - 128, NSEG], f32, tag="t2")
            nc.tensor.transpose(out=t2_ps[:, :], in_=a_sb[:, 128:FDIM],
                                identity=ident_t[:, :])
            at1_sb = sb2.tile([128, NSEG], f32, tag="at1")
            nc.vector.tensor_copy(out=at1_sb[:, :], in_=t1_ps[:, :])
            at2_sb = sb2.tile([FDIM - 128, NSEG], f32, tag="at2")
            nc.scalar.activation(out=at2_sb[:, :], in_=t2_ps[:, :],
                                 func=mybir.ActivationFunctionType.Copy,
                                 bias=0.0, scale=1.0)

            o_ps = psT.tile([NSEG, HID], f32, tag="o")
            nc.tensor.matmul(out=o_ps[:, :], lhsT=at1_sb[:, :], rhs=wa_t[:, :],
                             start=True, stop=False)
            nc.tensor.matmul(out=o_ps[:, :], lhsT=at2_sb[:, :], rhs=wb_t[:, :],
                             start=False, stop=True)
            o_sb = sb2.tile([NSEG, HID], f32, tag="o_sb")
            nc.vector.tensor_copy(out=o_sb[:, :], in_=o_ps[:, :])
            nc.sync.dma_start(out=out_d[:, :], in_=o_sb[:, :])

    nc.compile()
    return nc


def prepare_inputs(f_atoms, W, func2atom, mapping, n_cores=N_CORES):
    flat = func2atom.astype(np.int64).ravel()
    seg = np.repeat(mapping.astype(np.int64), func2atom.shape[1])
    valid = flat > 0
    atom = flat[valid] - 1
    seg = seg[valid]

    ref_atoms, inv = np.unique(atom, return_inverse=True)
    nref = len(ref_atoms)
    ntiles = (nref + n_cores * 128 - 1) // (n_cores * 128)
    rows_pad = ntiles * 128                   # rows per core
    nrows = n_cores * rows_pad

    cnt = np.bincount(inv * NSEG + seg, minlength=nrows * NSEG)
    cnt = cnt.reshape(nrows, NSEG)
    nibble = cnt.max() <= 15

    payload = np.zeros((nrows, FBYTES + (50 if nibble else NSEG)),
                       dtype=np.uint8)
    payload[:nref, :FBYTES] = (
        f_atoms[ref_atoms].astype(ml_dtypes.bfloat16).view(np.uint8))
    if nibble:
        c8 = cnt.astype(np.uint8)
        payload[:, FBYTES:] = c8[:, :50] | (c8[:, 50:] << 4)
    else:
        payload[:, FBYTES:] = cnt.astype(np.float32).astype(FP8).view(np.uint8)

    ident = np.eye(NSEG, dtype=np.float32)
    wmat = W.astype(np.float32)
    in_maps = []
    for c in range(n_cores):
        sl = slice(c * rows_pad, (c + 1) * rows_pad)
        in_maps.append({
            "mrg": payload[sl].reshape(128, ntiles, -1),
            "wmat": wmat,
            "ident": ident,
        })
    return in_maps, ntiles, nibble


_CACHE = {}


def kernel(f_atoms, W, func2atom, mapping, func_save_init, _trace=False):
    in_maps, ntiles, nibble = prepare_inputs(f_atoms, W, func2atom, mapping)
    key = (ntiles, nibble)
    if key not in _CACHE:
        _CACHE[key] = build_nc(ntiles, nibble)
    nc = _CACHE[key]
    res = run_bass_kernel_spmd(nc, in_maps, list(range(N_CORES)),
                               trace=_trace)
    partial = sum(r["out"] for r in res.results)
    out = func_save_init.astype(np.float32) + partial.astype(np.float32)
    if _trace:
        kernel.last_exec_time_ns = res.exec_time_ns
    return out


# revision 6
# speedup vs baseline: 4.0527x; 4.0527x over previous
"""Trainium2 Bass kernel for CMPNEncoder functional-group embedding (v5).

out = func_save_init + A @ W,  A[m,:] = sum_a count_m[a] * f_atoms[a,:].

Host prep: compact to the globally-referenced atom rows (~80% of all),
split them evenly across 8 cores, and merge each row's payload into ONE
p-major u8 stream: [133 bf16 feature bytes | 50 nibble-packed count
bytes] = 316 B/row, so every partition's chunk is a single contiguous
multi-KB DMA descriptor.

On device, per 128-row tile: vector/gpsimd unpack the count nibbles to
an fp8 e4m3 [128,100] tile (exact for counts <= 15, host-checked), the
tensor engine contracts counts^T @ features into a PSUM [100,133]
accumulator (fp8 lhsT x bf16 rhs), then A @ W runs on-device.  Host
sums the per-core [100,300] partials and adds func_save_init.
"""

import sys

sys.path.insert(0, "/opt/trn_rl_repo")

import ml_dtypes
import numpy as np

import concourse.bacc as bacc
import concourse.mybir as mybir
from concourse.bass_utils import run_bass_kernel_spmd
from concourse.tile import TileContext

N_ATOMS = 400_000
FDIM = 133
HID = 300
NSEG = 100
N_CORES = 8
CH = 16                                   # 128-row tiles per DMA chunk

FP8 = ml_dtypes.float8_e4m3fn
FBYTES = 2 * FDIM                         # 266 bf16 feature bytes per row


def _chunk_schedule(ntiles):
    """(t0, g) list: small chunks first so the first matmul starts early."""
    sched, t0 = [], 0
    for g in (4, 4, 8):
        if t0 + g > ntiles:
            break
        sched.append((t0, g))
        t0 += g
    while t0 < ntiles:
        g = min(CH, ntiles - t0)
        sched.append((t0, g))
        t0 += g
    return sched


def build_nc(ntiles, nibble=True):
    f32, bf16 = mybir.dt.float32, mybir.dt.bfloat16
    u8, fp8 = mybir.dt.uint8, mybir.dt.float8e4
    rowb = FBYTES + (50 if nibble else NSEG)    # 316 or 366 B per row

    nc = bacc.Bacc("TRN2", target_bir_lowering=False, debug=False)

    mrg = nc.declare_dram_parameter("mrg", [128, ntiles, rowb], u8,
                                    isOutput=False)
    wmat = nc.declare_dram_parameter("wmat", [FDIM, HID], f32, isOutput=False)
    ident_d = nc.declare_dram_parameter("ident", [NSEG, NSEG], f32,
                                        isOutput=False)
    out_d = nc.declare_dram_parameter("out", [NSEG, HID], f32, isOutput=True)

    sched = _chunk_schedule(ntiles)

    with TileContext(nc) as tc:
        with (
            tc.tile_pool(name="const", bufs=1) as cpool,
            tc.tile_pool(name="stream", bufs=8) as spool,
            tc.tile_pool(name="unp", bufs=8) as upool,
            tc.tile_pool(name="psA", bufs=1, space="PSUM") as psA,
            tc.tile_pool(name="psT", bufs=1, space="PSUM") as psT,
            tc.tile_pool(name="sb2", bufs=1) as sb2,
        ):
            st0 = []
            for ck, (t0, g) in enumerate(sched[:2]):
                st = spool.tile([128, CH, rowb], u8, tag="s")
                nc.sync.dma_start(out=st[:, 0:g, :], in_=mrg[:, t0:t0 + g, :])
                st0.append(st)

            ident_t = cpool.tile([NSEG, NSEG], f32, tag="ident")
            nc.sync.dma_start(out=ident_t[:, :], in_=ident_d[:, :])
            wa_t = cpool.tile([128, HID], f32, tag="wa")
            nc.sync.dma_start(out=wa_t[:, :], in_=wmat[0:128, :])
            wb_t = cpool.tile([FDIM - 128, HID], f32, tag="wb")
            nc.sync.dma_start(out=wb_t[:, :], in_=wmat[128:FDIM, :])

            a_ps = psA.tile([NSEG, FDIM], f32, tag="A")

            tglob = 0
            for ck, (t0, g) in enumerate(sched):
                if ck < 2:
                    st = st0[ck]
                else:
                    st = spool.tile([128, CH, rowb], u8, tag="s")
                    nc.sync.dma_start(out=st[:, 0:g, :],
                                      in_=mrg[:, t0:t0 + g, :])
                if nibble:
                    pk = st[:, 0:g, FBYTES:FBYTES + 50]
                    tu = upool.tile([128, CH, NSEG], mybir.dt.uint8, tag="tu")
                    nc.vector.tensor_scalar(
                        out=tu[:, 0:g, 0:50], in0=pk, scalar1=15, scalar2=None,
                        op0=mybir.AluOpType.bitwise_and)
                    nc.vector.tensor_scalar(
                        out=tu[:, 0:g, 50:100], in0=pk, scalar1=4, scalar2=None,
                        op0=mybir.AluOpType.logical_shift_right)
                    wu = upool.tile([128, CH, NSEG], fp8, tag="wu")
                    nc.gpsimd.tensor_scalar(
                        out=wu[:, 0:g, 0:50], in0=tu[:, 0:g, 0:50], scalar1=0,
                        scalar2=None, op0=mybir.AluOpType.add)
                    nc.scalar.activation(
                        out=wu[:, 0:g, 50:100], in_=tu[:, 0:g, 50:100],
                        func=mybir.ActivationFunctionType.Copy,
                        bias=0.0, scale=1.0)
                for j in range(g):
                    if nibble:
                        lhs = wu[:, j, :]
                    else:
                        lhs = st[:, j, FBYTES:FBYTES + NSEG].bitcast(fp8)
                    nc.tensor.matmul(
                        out=a_ps[:, :],
                        lhsT=lhs,
                        rhs=st[:, j, 0:FBYTES].bitcast(bf16),
                        start=(tglob == 0),
                        stop=(tglob == ntiles - 1),
                    )
                    tglob += 1

            a_sb = sb2.tile([NSEG, FDIM], f32, tag="a_sb")
            nc.vector.tensor_copy(out=a_sb[:, :], in_=a_ps[:, :])
            t1_ps = psT.tile([128, NSEG], f32, tag="t1")
            nc.tensor.transpose(out=t1_ps[:, :], in_=a_sb[:, 0:128],
                                identity=ident_t[:, :])
            t2_ps = psT.tile([FDIM - 128, NSEG], f32, tag="t2")
            nc.tensor.transpose(out=t2_ps[:, :], in_=a_sb[:, 128:FDIM],
                                identity=ident_t[:, :])
            at1_sb = sb2.tile([128, NSEG], f32, tag="at1")
            nc.vector.tensor_copy(out=at1_sb[:, :], in_=t1_ps[:, :])
            at2_sb = sb2.tile([FDIM - 128, NSEG], f32, tag="at2")
            nc.scalar.activation(out=at2_sb[:, :], in_=t2_ps[:, :],
                                 func=mybir.ActivationFunctionType.Copy,
                                 bias=0.0, scale=1.0)

            o_ps = psT.tile([NSEG, HID], f32, tag="o")
            nc.tensor.matmul(out=o_ps[:, :], lhsT=at1_sb[:, :], rhs=wa_t[:, :],
                             start=True, stop=False)
            nc.tensor.matmul(out=o_ps[:, :], lhsT=at2_sb[:, :], rhs=wb_t[:, :],
                             start=False, stop=True)
            o_sb = sb2.tile([NSEG, HID], f32, tag="o_sb")
            nc.vector.tensor_copy(out=o_sb[:, :], in_=o_ps[:, :])
            nc.sync.dma_start(out=out_d[:, :], in_=o_sb[:, :])

    nc.compile()
    return nc


def prepare_inputs(f_atoms, W, func2atom, mapping, n_cores=N_CORES):
    flat = func2atom.astype(np.int64).ravel()
    seg = np.repeat(mapping.astype(np.int64), func2atom.shape[1])
    valid = flat > 0
    atom = flat[valid] - 1
    seg = seg[valid]

    ref_atoms, inv = np.unique(atom, return_inverse=True)
    nref = len(ref_atoms)
    ntiles = (nref + n_cores * 128 - 1) // (n_cores * 128)
    rows_pad = ntiles * 128                   # rows per core
    nrows = n_cores * rows_pad

    cnt = np.bincount(inv * NSEG + seg, minlength=nrows * NSEG)
    cnt = cnt.reshape(nrows, NSEG)
    nibble = False                # device-side nibble unpack too slow (DVE)

    payload = np.zeros((nrows, FBYTES + (50 if nibble else NSEG)),
                       dtype=np.uint8)
    payload[:nref, :FBYTES] = (
        f_atoms[ref_atoms].astype(ml_dtypes.bfloat16).view(np.uint8))
    if nibble:
        c8 = cnt.astype(np.uint8)
        payload[:, FBYTES:] = c8[:, :50] | (c8[:, 50:] << 4)
    else:
        payload[:, FBYTES:] = cnt.astype(np.float32).astype(FP8).view(np.uint8)

    ident = np.eye(NSEG, dtype=np.float32)
    wmat = W.astype(np.float32)
    in_maps = []
    for c in range(n_cores):
        sl = slice(c * rows_pad, (c + 1) * rows_pad)
        in_maps.append({
            "mrg": payload[sl].reshape(128, ntiles, -1),
            "wmat": wmat,
            "ident": ident,
        })
    return in_maps, ntiles, nibble


_CACHE = {}


def kernel(f_atoms, W, func2atom, mapping, func_save_init, _trace=False):
    in_maps, ntiles, nibble = prepare_inputs(f_atoms, W, func2atom, mapping)
    key = (ntiles, nibble)
    if key not in _CACHE:
        _CACHE[key] = build_nc(ntiles, nibble)
    nc = _CACHE[key]
    res = run_bass_kernel_spmd(nc, in_maps, list(range(N_CORES)),
                               trace=_trace)
    partial = sum(r["out"] for r in res.results)
    out = func_save_init.astype(np.float32) + partial.astype(np.float32)
    if _trace:
        kernel.last_exec_time_ns = res.exec_time_ns
    return out


# revision 8
# speedup vs baseline: 4.5300x; 1.1178x over previous
"""Trainium2 Bass kernel for CMPNEncoder functional-group embedding (v5).

out = func_save_init + A @ W,  A[m,:] = sum_a count_m[a] * f_atoms[a,:].

Host prep: compact to the globally-referenced atom rows (~80% of all),
split them evenly across 8 cores, and merge each row's payload into ONE
p-major u8 stream: [133 bf16 feature bytes | 50 nibble-packed count
bytes] = 316 B/row, so every partition's chunk is a single contiguous
multi-KB DMA descriptor.

On device, per 128-row tile: vector/gpsimd unpack the count nibbles to
an fp8 e4m3 [128,100] tile (exact for counts <= 15, host-checked), the
tensor engine contracts counts^T @ features into a PSUM [100,133]
accumulator (fp8 lhsT x bf16 rhs), then A @ W runs on-device.  Host
sums the per-core [100,300] partials and adds func_save_init.
"""

import sys

sys.path.insert(0, "/opt/trn_rl_repo")

import ml_dtypes
import numpy as np

import concourse.bacc as bacc
import concourse.mybir as mybir
from concourse.bass_utils import run_bass_kernel_spmd
from concourse.tile import TileContext

N_ATOMS = 400_000
FDIM = 133
HID = 300
NSEG = 100
N_CORES = 8
CH = 16                                   # 128-row tiles per DMA chunk

FP8 = ml_dtypes.float8_e4m3fn
FBYTES = 2 * FDIM                         # 266 bf16 feature bytes per row


def _chunk_schedule(ntiles):
    """Uniform chunks; every chunk gets its own SBUF buffer (no recycle)."""
    sched, t0 = [], 0
    while t0 < ntiles:
        g = min(CH, ntiles - t0)
        sched.append((t0, g))
        t0 += g
    return sched


def build_nc(ntiles, nibble=True):
    f32, bf16 = mybir.dt.float32, mybir.dt.bfloat16
    u8, fp8 = mybir.dt.uint8, mybir.dt.float8e4
    rowb = FBYTES + (50 if nibble else NSEG)    # 316 or 366 B per row

    nc = bacc.Bacc("TRN2", target_bir_lowering=False, debug=False)

    mrg = nc.declare_dram_parameter("mrg", [128, ntiles, rowb], u8,
                                    isOutput=False)
    wmat = nc.declare_dram_parameter("wmat", [FDIM, HID], f32, isOutput=False)
    ident_d = nc.declare_dram_parameter("ident", [NSEG, NSEG], f32,
                                        isOutput=False)
    out_d = nc.declare_dram_parameter("out", [NSEG, HID], f32, isOutput=True)

    sched = _chunk_schedule(ntiles)

    with TileContext(nc) as tc:
        with (
            tc.tile_pool(name="const", bufs=1) as cpool,
            tc.tile_pool(name="stream", bufs=len(sched)) as spool,
            tc.tile_pool(name="unp", bufs=8) as upool,
            tc.tile_pool(name="psA", bufs=1, space="PSUM") as psA,
            tc.tile_pool(name="psT", bufs=1, space="PSUM") as psT,
            tc.tile_pool(name="sb2", bufs=1) as sb2,
        ):
            # The full stream fits in SBUF: issue every chunk's DMA up
            # front so the 16 queues run saturated with no recycle deps.
            sts = []
            for t0, g in sched:
                st = spool.tile([128, CH, rowb], u8, tag="s")
                nc.sync.dma_start(out=st[:, 0:g, :], in_=mrg[:, t0:t0 + g, :])
                sts.append(st)

            ident_t = cpool.tile([NSEG, NSEG], f32, tag="ident")
            nc.sync.dma_start(out=ident_t[:, :], in_=ident_d[:, :])
            wa_t = cpool.tile([128, HID], f32, tag="wa")
            nc.sync.dma_start(out=wa_t[:, :], in_=wmat[0:128, :])
            wb_t = cpool.tile([FDIM - 128, HID], f32, tag="wb")
            nc.sync.dma_start(out=wb_t[:, :], in_=wmat[128:FDIM, :])

            a_ps = psA.tile([NSEG, FDIM], f32, tag="A")

            tglob = 0
            for ck, (t0, g) in enumerate(sched):
                st = sts[ck]
                if nibble:
                    pk = st[:, 0:g, FBYTES:FBYTES + 50]
                    tu = upool.tile([128, CH, NSEG], mybir.dt.uint8, tag="tu")
                    nc.vector.tensor_scalar(
                        out=tu[:, 0:g, 0:50], in0=pk, scalar1=15, scalar2=None,
                        op0=mybir.AluOpType.bitwise_and)
                    nc.vector.tensor_scalar(
                        out=tu[:, 0:g, 50:100], in0=pk, scalar1=4, scalar2=None,
                        op0=mybir.AluOpType.logical_shift_right)
                    wu = upool.tile([128, CH, NSEG], fp8, tag="wu")
                    nc.gpsimd.tensor_scalar(
                        out=wu[:, 0:g, 0:50], in0=tu[:, 0:g, 0:50], scalar1=0,
                        scalar2=None, op0=mybir.AluOpType.add)
                    nc.scalar.activation(
                        out=wu[:, 0:g, 50:100], in_=tu[:, 0:g, 50:100],
                        func=mybir.ActivationFunctionType.Copy,
                        bias=0.0, scale=1.0)
                for j in range(g):
                    if nibble:
                        lhs = wu[:, j, :]
                    else:
                        lhs = st[:, j, FBYTES:FBYTES + NSEG].bitcast(fp8)
                    nc.tensor.matmul(
                        out=a_ps[:, :],
                        lhsT=lhs,
                        rhs=st[:, j, 0:FBYTES].bitcast(bf16),
                        start=(tglob == 0),
                        stop=(tglob == ntiles - 1),
                    )
                    tglob += 1

            a_sb = sb2.tile([NSEG, FDIM], f32, tag="a_sb")
            nc.vector.tensor_copy(out=a_sb[:, :], in_=a_ps[:, :])
            t1_ps = psT.tile([128, NSEG], f32, tag="t1")
            nc.tensor.transpose(out=t1_ps[:, :], in_=a_sb[:, 0:128],
                                identity=ident_t[:, :])
            t2_ps = psT.tile([FDIM - 128, NSEG], f32, tag="t2")
            nc.tensor.transpose(out=t2_ps[:, :], in_=a_sb[:, 128:FDIM],
                                identity=ident_t[:, :])
            at1_sb = sb2.tile([128, NSEG], f32, tag="at1")
            nc.vector.tensor_copy(out=at1_sb[:, :], in_=t1_ps[:, :])
            at2_sb = sb2.tile([FDIM - 128, NSEG], f32, tag="at2")
            nc.scalar.activation(out=at2_sb[:, :], in_=t2_ps[:, :],
                                 func=mybir.ActivationFunctionType.Copy,
                                 bias=0.0, scale=1.0)

            o_ps = psT.tile([NSEG, HID], f32, tag="o")
            nc.tensor.matmul(out=o_ps[:, :], lhsT=at1_sb[:, :], rhs=wa_t[:, :],
                             start=True, stop=False)
            nc.tensor.matmul(out=o_ps[:, :], lhsT=at2_sb[:, :], rhs=wb_t[:, :],
                             start=False, stop=True)
            o_sb = sb2.tile([NSEG, HID], f32, tag="o_sb")
            nc.vector.tensor_copy(out=o_sb[:, :], in_=o_ps[:, :])
            nc.sync.dma_start(out=out_d[:, :], in_=o_sb[:, :])

    nc.compile()
    return nc


def prepare_inputs(f_atoms, W, func2atom, mapping, n_cores=N_CORES):
    flat = func2atom.astype(np.int64).ravel()
    seg = np.repeat(mapping.astype(np.int64), func2atom.shape[1])
    valid = flat > 0
    atom = flat[valid] - 1
    seg = seg[valid]

    ref_atoms, inv = np.unique(atom, return_inverse=True)
    nref = len(ref_atoms)
    ntiles = (nref + n_cores * 128 - 1) // (n_cores * 128)
    rows_pad = ntiles * 128                   # rows per core
    nrows = n_cores * rows_pad

    cnt = np.bincount(inv * NSEG + seg, minlength=nrows * NSEG)
    cnt = cnt.reshape(nrows, NSEG)
    nibble = False                # device-side nibble unpack too slow (DVE)

    payload = np.zeros((nrows, FBYTES + (50 if nibble else NSEG)),
                       dtype=np.uint8)
    payload[:nref, :FBYTES] = (
        f_atoms[ref_atoms].astype(ml_dtypes.bfloat16).view(np.uint8))
    if nibble:
        c8 = cnt.astype(np.uint8)
        payload[:, FBYTES:] = c8[:, :50] | (c8[:, 50:] << 4)
    else:
        payload[:, FBYTES:] = cnt.astype(np.float32).astype(FP8).view(np.uint8)

    ident = np.eye(NSEG, dtype=np.float32)
    wmat = W.astype(np.float32)
    in_maps = []
    for c in range(n_cores):
        sl = slice(c * rows_pad, (c + 1) * rows_pad)
        in_maps.append({
            "mrg": payload[sl].reshape(128, ntiles, -1),
            "wmat": wmat,
            "ident": ident,
        })
    return in_maps, ntiles, nibble


_CACHE = {}


def kernel(f_atoms, W, func2atom, mapping, func_save_init, _trace=False):
    in_maps, ntiles, nibble = prepare_inputs(f_atoms, W, func2atom, mapping)
    key = (ntiles, nibble)
    if key not in _CACHE:
        _CACHE[key] = build_nc(ntiles, nibble)
    nc = _CACHE[key]
    res = run_bass_kernel_spmd(nc, in_maps, list(range(N_CORES)),
                               trace=_trace)
    partial = sum(r["out"] for r in res.results)
    out = func_save_init.astype(np.float32) + partial.astype(np.float32)
    if _trace:
        kernel.last_exec_time_ns = res.exec_time_ns
    return out
